# revision 32
# baseline (speedup 1.0000x reference)
"""Trainium2 Bass kernel for the shared-weight transformer encoder with a
Conv1d-ensemble FFN (nn_MCAT_23630910062939).

Sharding: data-parallel over batch - each of the 8 NeuronCores computes one
full batch element; no collectives.  Residual stream lives on-chip in
feature-major layout x^T [D, S] (bf16); all matmuls run in fp8e4m3 with
MatmulPerfMode.DoubleRow (contraction pairs packed along a free dim), with
per-tensor power-of-two quantization scales folded into the surrounding
bias/scale ops.  Softmax exp is the ACT-engine critical path; the emission
order overlaps conv/LN/projection work (PE/DVE/Pool) under the exp stream,
including cross-layer overlap via query-half splitting.
"""
import sys
sys.path.insert(0, '/opt/trn_rl_repo')
import math
import numpy as np
import ml_dtypes

from contextlib import ExitStack
import concourse.bass as bass
import concourse.mybir as mybir
import concourse.tile as tile
from concourse import bacc, library_config
from concourse.bass_utils import run_bass_kernel_spmd
from concourse.dve_ops import RECIPROCAL_APPROX_NR

P = 128
D = 1024
S = 1024
H = 16
DK = 64
CH = 8            # 128-row feature chunks
CP = 4            # 256-row chunk pairs (DoubleRow)
NH = 2            # 512-col token halves
NL = 2
N_CORES = 8
EPS_BN = 1e-5
MAGIC = 0x5f3759df

f32 = mybir.dt.float32
i32 = mybir.dt.int32
bf16 = mybir.dt.bfloat16
fp8 = mybir.dt.float8e4
AF = mybir.ActivationFunctionType
OP = mybir.AluOpType
PM = mybir.MatmulPerfMode
E4 = ml_dtypes.float8_e4m3

# branch -> taps in cw pack order: (pack_idx, shift); taps 9-17 hold the
# fp8 hi/lo residual weights (same quant scale, accumulated in-chain)
BRANCH_TAPS = [
    [(0, -2), (1, -1), (2, 0), (3, 1), (4, 2),
     (9, -2), (10, -1), (11, 0), (12, 1), (13, 2)],   # filter 5 hi+lo
    [(5, -1), (6, 0), (7, 1), (14, -1), (15, 0), (16, 1)],  # filter 3 hi+lo
    [(8, 0), (17, 0)],                                      # filter 1 hi+lo
]
N_TAPS = 18


def _build(sc, debug=False):
    """sc: dict with quant scales kq, kk, kv, ko, kc (floats, powers of 2)."""
    nc = bacc.Bacc(None, target_bir_lowering=False)
    names = {}

    def reg(t, key):
        names[key] = t.name
        return t

    iq, ik, iv, io, ic_ = (1.0 / sc['kq'], 1.0 / sc['kk'], 1.0 / sc['kv'],
                           1.0 / sc['ko'], 1.0 / sc['kc'])

    with tile.TileContext(nc) as tc, ExitStack() as stack:
        with tc.tile_pool(name="dram", bufs=1, space="DRAM") as dram:
            xt_d = reg(dram.tile([D, S], bf16, kind="ExternalInput", name="xt"), "xt")
            wq_d = reg(dram.tile([P, CH, CP, 2, P], fp8, kind="ExternalInput", name="wq"), "wq")
            wk_d = reg(dram.tile([P, CH, CP, 2, P], fp8, kind="ExternalInput", name="wk"), "wk")
            wo_d = reg(dram.tile([P, CH, CP, 2, P], fp8, kind="ExternalInput", name="wo"), "wo")
            wv_d = reg(dram.tile([P, CP, 2, D], fp8, kind="ExternalInput", name="wv"), "wv")
            cw_d = reg(dram.tile([N_TAPS, CH, P, CP, 2, P], fp8, kind="ExternalInput", name="cw"), "cw")
            bq_d = reg(dram.tile([P, CH], f32, kind="ExternalInput", name="bq"), "bq")
            bk_d = reg(dram.tile([P, CH], f32, kind="ExternalInput", name="bk"), "bk")
            bo_d = reg(dram.tile([P, CH], f32, kind="ExternalInput", name="bo"), "bo")
            bv_d = reg(dram.tile([1, D], f32, kind="ExternalInput", name="bv"), "bv")
            cb_d = reg(dram.tile([P, 3, CH], f32, kind="ExternalInput", name="cb"), "cb")
            icc_d = reg(dram.tile([P, CH], f32, kind="ExternalInput", name="icc"), "icc")
            yt_d = reg(dram.tile([D, S], f32, kind="ExternalOutput", name="yt"), "yt")

            def dbg(key, ap, shape, dtype):
                if not debug:
                    return
                t = dram.tile(shape, dtype, kind="ExternalOutput", name=f"dbg_{key}")
                names[f"dbg_{key}"] = t.name
                nc.sync.dma_start(t[:], ap)

            nc.gpsimd.load_library(library_config.attn)

            glob = stack.enter_context(tc.tile_pool(name="glob", bufs=1))
            rot = stack.enter_context(tc.tile_pool(name="rot", bufs=1))
            psum = stack.enter_context(tc.tile_pool(name="psum", bufs=1, space="PSUM"))

            # ---------------- persistent tiles ----------------
            x = glob.tile([P, CH, S], bf16, tag="x")
            act1 = glob.tile([P, CH, S], fp8, tag="act1")
            act2 = glob.tile([P, CH, S + 4], fp8, tag="act2")
            qt = glob.tile([P, 4, 2, S], fp8, tag="qt")    # [32-group, h//4, dkslot, key/query tok]
            kt = glob.tile([P, 4, 2, S], fp8, tag="kt")
            ot = glob.tile([P, CH, S], fp8, tag="ot")
            wqr = glob.tile([P, CH, CP, 2, P], fp8, tag="wqr")
            wkr = glob.tile([P, CH, CP, 2, P], fp8, tag="wkr")
            wor = glob.tile([P, CH, CP, 2, P], fp8, tag="wor")
            vwr = glob.tile([P, CP, 2, D], fp8, tag="vwr")
            bq_sb = glob.tile([P, CH], f32, tag="bq")
            bk_sb = glob.tile([P, CH], f32, tag="bk")
            bo_sb = glob.tile([P, CH], f32, tag="bo")
            cb_sb = glob.tile([P, 3, CH], f32, tag="cb")
            icc_sb = glob.tile([P, CH], f32, tag="icc")
            bv_row = glob.tile([1, D], f32, tag="bvr")
            bv_b = glob.tile([P, D], f32, tag="bvb")
            ones_bf = glob.tile([P, 1], bf16, tag="ones")
            zeros = glob.tile([P, 512], bf16, tag="zeros")
            rowbuf0 = glob.tile([1, 512], f32, tag="rowbuf0")
            rowbuf1 = glob.tile([1, 512], f32, tag="rowbuf1")
            rowbuf2 = glob.tile([1, 512], f32, tag="rowbuf2")
            rowbuf3 = glob.tile([1, 512], f32, tag="rowbuf3")

            # ---------------- prelude ----------------
            nc.sync.dma_start(wqr[:], wq_d[:])
            nc.sync.dma_start(wkr[:], wk_d[:])
            nc.sync.dma_start(wor[:], wo_d[:])
            nc.sync.dma_start(vwr[:], wv_d[:])
            nc.sync.dma_start(bq_sb[:], bq_d[:])
            nc.sync.dma_start(bk_sb[:], bk_d[:])
            nc.sync.dma_start(bo_sb[:], bo_d[:])
            nc.sync.dma_start(cb_sb[:], cb_d[:])
            nc.sync.dma_start(icc_sb[:], icc_d[:])
            nc.sync.dma_start(bv_row[:], bv_d[:])
            for c in range(CH):
                nc.sync.dma_start(x[:, c, :], xt_d[c * P:(c + 1) * P, :])
            nc.gpsimd.partition_broadcast(bv_b[:], bv_row[:])
            nc.vector.memset(ones_bf[:], 1.0)
            nc.vector.memset(zeros[:], 0.0)
            nc.vector.memset(act2[:, :, 0:2], 0.0)
            nc.vector.memset(act2[:, :, S + 2:S + 4], 0.0)

            # vt: per layer (bufs=2) [P, tokchunk, h*65]; ones col set per alloc
            vt_of = {}

            def new_vt():
                t = rot.tile([P, CH, H * 80], fp8, tag="vt", bufs=2)
                v5 = t[:].rearrange("p c (h e) -> p c h e", e=80)
                nc.vector.memset(v5[:, :, :, 64:65], 1.0)
                nc.gpsimd.memset(v5[:, :, :, 65:80], 0.0)
                return t

            # ---------------- helpers ----------------
            def half_cols(n):
                return slice(n * 512, (n + 1) * 512)

            def ln_stats(n):
                """mean/msq chains for token-half n -> work-pool psum tile
                with sum(x) at row 0, sum(x^2) at row 32."""
                cols = half_cols(n)
                st = psum.tile([P, 512], f32, tag="work", bufs=2)
                for c in range(CH):
                    sq = rot.tile([P, 512], bf16, tag="sq", bufs=2)
                    nc.vector.tensor_tensor(sq[:], x[:, c, cols], x[:, c, cols], OP.mult)
                    nc.tensor.matmul(st[0:1, :], ones_bf[:], x[:, c, cols],
                                     start=(c == 0), stop=(c == CH - 1))
                    nc.tensor.matmul(st[32:33, :], ones_bf[:], sq[:],
                                     start=(c == 0), stop=(c == CH - 1))
                return st

            def ln_rows(st):
                """-> (inv_b, minv_b) [P,512] bf16 broadcasts from stats tile."""
                # four rotating row slots: r0 (t, y1), r1 (v1), r2 (h, z, z2),
                # r3 (s, inv_f)
                t = rowbuf0[:]
                nc.scalar.activation(t, st[0:1, :], AF.Square)
                v1 = rowbuf1[:]
                nc.vector.scalar_tensor_tensor(v1, t, -1.0 / D,
                                               st[32:33, :],
                                               op0=OP.mult, op1=OP.add)
                # rsqrt(v1/(D-1)) via bit-trick seed with the /(D-1) ~= 2^-10
                # folded into the magic (+5 exponent halves) and the NR scale
                h_ = rowbuf2[:]
                nc.vector.tensor_scalar(h_.bitcast(i32), v1.bitcast(i32), 1, None,
                                        op0=OP.logical_shift_right)
                s_ = rowbuf3[:]
                nc.vector.tensor_scalar(s_.bitcast(i32), h_.bitcast(i32), MAGIC + 5 * 0x800000, -1,
                                        op0=OP.subtract, op1=OP.mult)
                z_ = rowbuf2[:]
                nc.vector.scalar_tensor_tensor(z_, v1, 0.5 / (D - 1), s_, op0=OP.mult, op1=OP.mult)
                y1 = rowbuf0[:]
                nc.vector._custom_dve(RECIPROCAL_APPROX_NR, out=y1, in0=z_, in1=s_, s0=1.5)
                z2 = rowbuf2[:]
                nc.vector.scalar_tensor_tensor(z2, v1, 0.5 / (D - 1), y1, op0=OP.mult, op1=OP.mult)
                inv_f = rowbuf3[:]
                nc.vector._custom_dve(RECIPROCAL_APPROX_NR, out=inv_f, in0=z2, in1=y1, s0=1.5)
                inv_bf = rot.tile([1, 512], bf16, tag="invr", bufs=1)
                nc.vector.tensor_scalar(inv_bf[:], inv_f, 1.0, None, op0=OP.mult)
                minv_bf = rot.tile([1, 512], bf16, tag="minvr", bufs=1)
                nc.vector.scalar_tensor_tensor(minv_bf[:], st[0:1, :],
                                               1.0 / D, inv_f, op0=OP.mult, op1=OP.mult)
                inv_b = rot.tile([P, 512], bf16, tag="invb", bufs=2)
                minv_b = rot.tile([P, 512], bf16, tag="minvb", bufs=2)
                nc.gpsimd.partition_broadcast(inv_b[:], inv_bf[:])
                nc.gpsimd.partition_broadcast(minv_b[:], minv_bf[:])
                return inv_b, minv_b

            def ln_apply(n, inv_b, minv_b, dst, pad):
                """normalize x half n -> dst (fp8, pad offset), DVE mul + Pool sub."""
                cols = half_cols(n)
                for c in range(CH):
                    t = rot.tile([P, 512], bf16, tag="lnt", bufs=2)
                    nc.vector.tensor_tensor(t[:], x[:, c, cols], inv_b[:], OP.mult)
                    nc.gpsimd.tensor_tensor(dst[:, c, pad + n * 512:pad + (n + 1) * 512],
                                            t[:], minv_b[:], OP.subtract)

            def lnf_apply(n, inv_b, minv_b):
                """final LN half n -> yt (f32) via DVE, DMA out."""
                cols = half_cols(n)
                for c in range(CH):
                    t = rot.tile([P, 512], bf16, tag="lnt", bufs=2)
                    nc.vector.tensor_tensor(t[:], x[:, c, cols], inv_b[:], OP.mult)
                    o = rot.tile([P, 512], f32, tag="yts", bufs=2)
                    nc.vector.tensor_tensor(o[:], t[:], minv_b[:], OP.subtract)
                    nc.sync.dma_start(yt_d[c * P:(c + 1) * P, cols], o[:])

            def qkproj(m, n, on_act=False):
                """Q and K projections for out-chunk m, token-half n; writes
                qt/kt via fp8 staging + partition-shuffle DMAs."""
                cols = half_cols(n)
                for (wres, bias, dstT, ksc) in ((wqr, bq_sb, qt, iq), (wkr, bk_sb, kt, ik)):
                    ps = psum.tile([P, 512], f32, tag="work", bufs=2)
                    for cp in range(CP):
                        nc.tensor.matmul(ps[:], wres[:, m, cp, :, :],
                                         act1[:, 2 * cp:2 * cp + 2, cols],
                                         start=(cp == 0), stop=(cp == CP - 1),
                                         perf_mode=PM.DoubleRow)
                    stg = rot.tile([P, 512], fp8, tag="stg", bufs=2)
                    if on_act:
                        nc.scalar.activation(stg[:], ps[:], AF.Identity,
                                             bias=bias[:, m:m + 1], scale=ksc)
                    else:
                        nc.vector.tensor_scalar(stg[:], ps[:], ksc, bias[:, m:m + 1],
                                                op0=OP.mult, op1=OP.add)
                    base = 64 * (m % 2)
                    nc.sync.dma_start(dstT[base:base + 64, m // 2, 0, cols], stg[0:64, :])
                    nc.sync.dma_start(dstT[base:base + 64, m // 2, 1, cols], stg[64:128, :])

            def vproj(vt, m, fh):
                """V projection token-chunk m, feature-half fh -> vt."""
                ps = psum.tile([P, 512], f32, tag="work", bufs=2)
                for cp in range(CP):
                    nc.tensor.matmul(ps[:], act1[:, 2 * cp:2 * cp + 2, m * P:(m + 1) * P],
                                     vwr[:, cp, :, 512 * fh:512 * (fh + 1)],
                                     start=(cp == 0), stop=(cp == CP - 1),
                                     perf_mode=PM.DoubleRow)
                v5 = vt[:].rearrange("p c (h e) -> p c h e", e=80)
                dst = v5[:, m, 8 * fh:8 * (fh + 1), 0:64]
                src = ps[:].rearrange("p (h e) -> p h e", e=64)
                bvv = bv_b[:, 512 * fh:512 * (fh + 1)].rearrange("p (h e) -> p h e", e=64)
                nc.vector.scalar_tensor_tensor(dst, src, iv, bvv, op0=OP.mult, op1=OP.add)

            def scores_exp(h, n, kclo, kchi, pexp_t):
                """score matmuls + exp for head h, query-half n, key chunks
                [kclo, kchi); exp into half-width pexp_t[:, kc-kclo, :] (fp8)."""
                g = h % 4
                hs = h // 4
                qcols = half_cols(n)
                for kc2 in range(kclo, kchi, 2):
                    scps = psum.tile([P, 2, 512], f32, tag="scps", bufs=2)
                    for j in range(2):
                        kc = kc2 + j
                        nc.tensor.matmul(scps[:, j, :],
                                         kt[32 * g:32 * g + 32, hs, :, kc * P:(kc + 1) * P],
                                         qt[32 * g:32 * g + 32, hs, :, qcols],
                                         start=True, stop=True, perf_mode=PM.DoubleRow,
                                         tile_position=(32 * g, 0))
                    ko_ = kc2 - kclo
                    nc.scalar.activation(pexp_t[:, ko_:ko_ + 2, :], scps[:], AF.Exp)

            def av_norm(vt, h, n, pexp_lo, pexp_hi):
                """AV + denominator + normalize -> ot for head h, q-half n."""
                cols = half_cols(n)
                av = psum.tile([P, 512], f32, tag="misc", bufs=2)
                for cp in range(CP):
                    pt = pexp_lo if cp < 2 else pexp_hi
                    po = 2 * cp if cp < 2 else 2 * (cp - 2)
                    nc.tensor.matmul(av[0:80, :],
                                     vt[:, 2 * cp:2 * cp + 2, 80 * h:80 * h + 80],
                                     pt[:, po:po + 2, :],
                                     start=(cp == 0), stop=(cp == CP - 1),
                                     perf_mode=PM.DoubleRow)
                rrow = rot.tile([1, 512], f32, tag="rrow", bufs=1)
                nc.vector.reciprocal(rrow[:], av[64:65, :])
                rb = rot.tile([64, 512], f32, tag="rb", bufs=2)
                nc.gpsimd.partition_broadcast(rb[:], rrow[:])
                off = 64 * (h % 2)
                if debug and h == 0 and n == 0:
                    avc = rot.tile([P, 512], f32, tag="yts", bufs=2)
                    nc.vector.tensor_copy(avc[0:65, :], av[0:65, :])
                    dbg("av0", avc[:], [P, 512], f32)
                    dbg("rrow0", rrow[:], [1, 512], f32)
                nc.vector.tensor_tensor(ot[off:off + 64, h // 2, cols],
                                        av[0:64, :], rb[:], OP.mult)

            def wo_m(m, n):
                """output projection chunk m, token-half n + residual add."""
                cols = half_cols(n)
                ps = psum.tile([P, 512], f32, tag="work", bufs=2)
                for cp in range(CP):
                    nc.tensor.matmul(ps[:], wor[:, m, cp, :, :], ot[:, 2 * cp:2 * cp + 2, cols],
                                     start=(cp == 0), stop=(cp == CP - 1),
                                     perf_mode=PM.DoubleRow)
                tmp = rot.tile([P, 512], bf16, tag="wotmp", bufs=2)
                nc.scalar.activation(tmp[:], ps[:], AF.Identity,
                                     bias=bo_sb[:, m:m + 1], scale=io)
                nc.vector.tensor_tensor(x[:, m, cols], tmp[:], x[:, m, cols], OP.add)

            def conv_m(m, lo, hi, relu_act=False):
                """conv ensemble for out-chunk m, output cols [lo, hi)."""
                w = hi - lo
                scs = []
                for bi, taps in enumerate(BRANCH_TAPS):
                    cps = psum.tile([P, 512], f32, tag="work", bufs=2)
                    first = True
                    for (tp, shift) in taps:
                        wcv = rot.tile([P, CP, 2, P], fp8, tag="wcv", bufs=5)
                        nc.sync.dma_start(wcv[:], cw_d[tp, m])
                        u0 = 2 + shift + lo
                        for cp in range(CP):
                            nc.tensor.matmul(cps[:, 0:w], wcv[:, cp, :, :],
                                             act2[:, 2 * cp:2 * cp + 2, u0:u0 + w],
                                             start=first, stop=(cp == CP - 1 and tp == taps[-1][0]),
                                             perf_mode=PM.DoubleRow)
                            first = False
                    sc_t = rot.tile([P, 512], bf16, tag=f"sc{bi}", bufs=2)
                    if relu_act:
                        nc.scalar.activation(sc_t[:, 0:w], cps[:, 0:w], AF.Relu,
                                             bias=cb_sb[:, bi, m:m + 1], scale=1.0)
                    else:
                        nc.vector.scalar_tensor_tensor(sc_t[:, 0:w], cps[:, 0:w], cb_sb[:, bi, m:m + 1],
                                                       zeros[:, 0:w], op0=OP.add, op1=OP.max)
                    scs.append(sc_t)
                t1 = rot.tile([P, 512], bf16, tag="cmb1", bufs=1)
                nc.gpsimd.tensor_tensor(t1[:, 0:w], scs[0][:, 0:w], scs[1][:, 0:w], OP.add)
                t2 = rot.tile([P, 512], bf16, tag="cmb2", bufs=1)
                nc.gpsimd.tensor_tensor(t2[:, 0:w], t1[:, 0:w], scs[2][:, 0:w], OP.add)
                nc.vector.scalar_tensor_tensor(x[:, m, lo:hi], t2[:, 0:w], icc_sb[:, m:m + 1],
                                               x[:, m, lo:hi], op0=OP.mult, op1=OP.add)

            # ---------------- PSUM layout ----------------

            # ---------------- emission ----------------
            # Interleaved schedule: while ACT streams exp for one query-half,
            # PE runs the other half's AV/conv/LN/projections and the next
            # layer's lead-in.  pexp is split into lo/hi key-half tiles;
            # pxAlo has bufs=H because the next layer's lo-exps (p1) stay
            # live until the AVs in the p2 loop.
            pex = {}

            def new_px(key, tag, bufs):
                t = rot.tile([P, CH // 2, 512], fp8, tag=tag, bufs=bufs, name="px")
                pex[key] = t
                return t

            # ---- layer-0 lead-in ----
            stA = ln_stats(0)
            stB = ln_stats(1)
            iA, mA = ln_rows(stA)
            ln_apply(0, iA, mA, act1, 0)
            iB, mB = ln_rows(stB)
            ln_apply(1, iB, mB, act1, 0)
            dbg("act1", act1[:], [P, CH, S], fp8)
            dbg("ivA", iA[:], [P, 512], bf16)
            dbg("mvA", mA[:], [P, 512], bf16)
            for m in range(CH):
                qkproj(m, 0, on_act=True)
                qkproj(m, 1, on_act=True)
            dbg("qt", qt[:], [P, 4, 2, S], fp8)
            dbg("kt", kt[:], [P, 4, 2, S], fp8)
            vt_of[0] = new_vt()
            for m in range(CH):
                for fh in range(2):
                    vproj(vt_of[0], m, fh)
            dbg("vt", vt_of[0][:], [P, CH, H * 80], fp8)
            # attn A of layer 0, AV lag-2
            for h in range(H):
                scores_exp(h, 0, 0, 4, new_px((0, 0, h, 0), "pxAlo", H))
                scores_exp(h, 0, 4, 8, new_px((0, 0, h, 1), "pxAhi", 2))
                if h == 0:
                    dbg("pexlo0", pex[(0, 0, 0, 0)][:], [P, CH // 2, 512], fp8)
                    dbg("pexhi0", pex[(0, 0, 0, 1)][:], [P, CH // 2, 512], fp8)
                if h >= 1:
                    av_norm(vt_of[0], h - 1, 0, pex[(0, 0, h - 1, 0)], pex[(0, 0, h - 1, 1)])
            av_norm(vt_of[0], H - 1, 0, pex[(0, 0, H - 1, 0)], pex[(0, 0, H - 1, 1)])
            dbg("otA", ot[:], [P, CH, S], fp8)
            for m in range(CH):
                wo_m(m, 0)
            dbg("xA", x[:], [P, CH, S], bf16)
            st2A = ln_stats(0)
            i2A, m2A = ln_rows(st2A)
            ln_apply(0, i2A, m2A, act2, 2)

            for l in range(NL):
                last = (l == NL - 1)
                # ---- B loop: scores/exp B + AV-B (lag-1) + conv-A cols [0:510)
                for h in range(H):
                    scores_exp(h, 1, 0, 4, new_px((l, 1, h, 0), "pxBlo", 2))
                    scores_exp(h, 1, 4, 8, new_px((l, 1, h, 1), "pxBhi", 2))
                    if h >= 1:
                        av_norm(vt_of[l], h - 1, 1, pex[(l, 1, h - 1, 0)], pex[(l, 1, h - 1, 1)])
                    if 1 <= h <= 8:
                        conv_m(h - 1, 0, 480)
                av_norm(vt_of[l], H - 1, 1, pex[(l, 1, H - 1, 0)], pex[(l, 1, H - 1, 1)])
                for m in range(CH):
                    wo_m(m, 1)
                st2B = ln_stats(1)
                i2B, m2B = ln_rows(st2B)
                ln_apply(1, i2B, m2B, act2, 2)

                if not last:
                    # ---- A' p1 loop: conv seam + chunk2, LN1'A, proj'A,
                    # scores'/exp' keys 0:4 (all 16 pexp-lo tiles stay live)
                    vt_of[l + 1] = new_vt()
                    for h in range(H):
                        if h == 0:
                            for mm_ in range(CH):
                                conv_m(mm_, 480, 512)      # seam cols (needs act2 B edge)
                            st1A = ln_stats(0)
                            i1A, m1A = ln_rows(st1A)
                            ln_apply(0, i1A, m1A, act1, 0)
                        if h < CH:
                            qkproj(h, 0)
                        # scores for head h need qt'/kt' chunk m=h//2 <= h (emitted)
                        scores_exp(h, 0, 0, 4, new_px((l + 1, 0, h, 0), "pxAlo", H))
                        if 2 <= h <= 9:
                            conv_m(h - 2, 512, S)          # cols [512:1024)
                        elif h == 10 or h == 11:
                            for mm_ in (2 * (h - 10), 2 * (h - 10) + 1):
                                vproj(vt_of[l + 1], mm_, 0)
                                vproj(vt_of[l + 1], mm_, 1)
                    # ---- A' p2 loop: LN1'B + proj'B + vproj'4-7 + keys 4:8 + AV'
                    for h in range(H):
                        if h == 0:
                            st1B = ln_stats(1)
                            i1B, m1B = ln_rows(st1B)
                            ln_apply(1, i1B, m1B, act1, 0)
                        if h < CH:
                            qkproj(h, 1)
                        if h == 0:
                            for mm_ in range(4, CH):
                                vproj(vt_of[l + 1], mm_, 0)
                                vproj(vt_of[l + 1], mm_, 1)
                        scores_exp(h, 0, 4, 8, new_px((l + 1, 0, h, 1), "pxAhi", 2))
                        if h >= 1:
                            av_norm(vt_of[l + 1], h - 1, 0,
                                    pex[(l + 1, 0, h - 1, 0)], pex[(l + 1, 0, h - 1, 1)])
                    av_norm(vt_of[l + 1], H - 1, 0, pex[(l + 1, 0, H - 1, 0)], pex[(l + 1, 0, H - 1, 1)])
                    for m in range(CH):
                        wo_m(m, 0)
                    st2A_ = ln_stats(0)
                    i2A_, m2A_ = ln_rows(st2A_)
                    ln_apply(0, i2A_, m2A_, act2, 2)
                else:
                    # ---- final: conv seam + chunk2 + final LN
                    for m in range(CH):
                        conv_m(m, 480, 512, relu_act=True)
                    stfA = ln_stats(0)
                    ifA, mfA = ln_rows(stfA)
                    lnf_apply(0, ifA, mfA)
                    for m in range(CH):
                        conv_m(m, 512, S, relu_act=True)
                    stfB = ln_stats(1)
                    ifB, mfB = ln_rows(stfB)
                    lnf_apply(1, ifB, mfB)

    nc.compile()
    return nc, names


_BUILT = None


def _pow2_scale(arr, target=192.0):
    am = float(np.abs(arr).max())
    if am <= 0:
        return 1.0
    return 2.0 ** math.floor(math.log2(target / am))


def _pack_dr(W, k):
    """W [D_in, D_out] (already folded+scaled by caller except k) ->
    [P, CH_m, CP, 2, P] fp8 with element [p, m, cp, i, j] = k*W[256cp+128i+p, 128m+j]."""
    Wq = (W * k).astype(E4)
    return np.ascontiguousarray(
        Wq.reshape(CP, 2, P, CH, P).transpose(2, 3, 0, 1, 4))


# local column permutation inside each 128-out-chunk for Q/K so that the
# staging rows land shuffle-DMA-contiguous: [a_dk0:32, b_dk0:32, a_dk32:64, b_dk32:64]
_QK_PERM = np.concatenate([np.arange(0, 32), np.arange(64, 96),
                           np.arange(32, 64), np.arange(96, 128)])


def _perm_qk_cols(W):
    Wc = W.reshape(D, CH, P)[:, :, _QK_PERM]
    return np.ascontiguousarray(Wc.reshape(D, D))


def _perm_qk_bias(b):
    return np.ascontiguousarray(b.reshape(CH, P)[:, _QK_PERM].reshape(D))


def _pack_bias(b):
    return np.ascontiguousarray(b.reshape(CH, P).T)


def _prep(inputs):
    f = lambda kk: np.asarray(inputs[kk], np.float32)
    a1, b1 = f('ln1_a'), f('ln1_b')
    a2, b2 = f('ln2_a'), f('ln2_b')
    wq, wk, wv, wo = f('wq'), f('wk'), f('wv'), f('wo')
    bq, bk, bv, bo = f('bq'), f('bk'), f('bv'), f('bo')

    wq_e = _perm_qk_cols(a1[:, None] * wq / 8.0)
    bq_e = _perm_qk_bias((bq + b1 @ wq) / 8.0)
    wk_e = _perm_qk_cols(a1[:, None] * wk)
    bk_e = _perm_qk_bias(bk + b1 @ wk)
    wv_e = a1[:, None] * wv
    bv_e = bv + b1 @ wv

    sc = {'kq': _pow2_scale(wq_e), 'kk': _pow2_scale(wk_e),
          'kv': _pow2_scale(wv_e), 'ko': _pow2_scale(wo)}

    d = {}
    d['wq'] = _pack_dr(wq_e, sc['kq'])
    d['bq'] = _pack_bias(bq_e)
    d['wk'] = _pack_dr(wk_e, sc['kk'])
    d['bk'] = _pack_bias(bk_e)
    # V: moving-side pack [P, CP, 2, D]: [p, cp, i, f] = kv*Wv[256cp+128i+p, f]
    d['wv'] = np.ascontiguousarray(
        (wv_e * sc['kv']).astype(E4).reshape(CP, 2, P, D).transpose(2, 0, 1, 3))
    d['bv'] = bv_e.reshape(1, D)
    d['wo'] = _pack_dr(wo, sc['ko'])
    d['bo'] = _pack_bias(bo)

    # conv: fold BN + ensemble mean + ln2 affine
    Wf_all, bias_all = [], []
    for bi, fs in enumerate((5, 3, 1)):
        i = 3 - bi
        W = f(f'conv_w{i}')
        b = f(f'conv_b{i}')
        g, beta = f(f'bn_g{i}'), f(f'bn_b{i}')
        m, v = f(f'bn_m{i}'), f(f'bn_v{i}')
        s = g / np.sqrt(v + EPS_BN)
        Wf_all.append(W * s[:, None, None] * a2[None, :, None] / 3.0)
        bias_all.append((b + W.sum(axis=2) @ b2 - m) * s + beta)
    bias_all = [bb / 3.0 for bb in bias_all]
    sc['kc'] = 1.0  # per-channel scales via icc instead
    # per-output-channel pow2 scale over all branches/taps
    am = np.max(np.stack([np.abs(Wf).max(axis=(1, 2)) for Wf in Wf_all]), axis=0)
    kvec = 2.0 ** np.floor(np.log2(192.0 / np.maximum(am, 1e-20)))

    cw = np.empty((N_TAPS, CH, P, CP, 2, P), E4)
    cb = np.empty((P, 3, CH), np.float32)
    for bi in range(3):
        cb[:, bi, :] = _pack_bias(bias_all[bi] * kvec)
        ntap = len(BRANCH_TAPS[bi]) // 2
        for j in range(ntap):
            tp_hi = BRANCH_TAPS[bi][j][0]
            tp_lo = BRANCH_TAPS[bi][ntap + j][0]
            Wk = np.ascontiguousarray(Wf_all[bi][:, :, j].T) * kvec[None, :]
            hi8 = Wk.astype(E4)
            resid = Wk - hi8.astype(np.float32)
            cw[tp_hi] = _pack_dr(hi8.astype(np.float32), 1.0).transpose(1, 0, 2, 3, 4)
            cw[tp_lo] = _pack_dr(resid, 1.0).transpose(1, 0, 2, 3, 4)
    d['cw'] = cw
    d['cb'] = cb
    d['icc'] = _pack_bias(1.0 / kvec)
    return d, sc


def kernel(**inputs):
    global _BUILT
    shared, sc = _prep(inputs)
    if _BUILT is None:
        _BUILT = _build(sc)
    nc, names = _BUILT
    x = np.asarray(inputs['x'], np.float32)
    in_maps = []
    for b in range(N_CORES):
        m = {names[k]: v for k, v in shared.items()}
        m[names['xt']] = np.ascontiguousarray(x[b].T).astype(ml_dtypes.bfloat16)
        in_maps.append(m)
    res = run_bass_kernel_spmd(nc, in_maps, core_ids=list(range(N_CORES)))
    af = np.asarray(inputs['lnf_a'], np.float32)
    bf = np.asarray(inputs['lnf_b'], np.float32)
    out = np.empty((N_CORES, S, D), np.float32)
    for b in range(N_CORES):
        yt = res.results[b][names['yt']]
        out[b] = yt.T * af[None, :] + bf[None, :]
    return out


# revision 38
# speedup vs baseline: 1.1992x; 1.1992x over previous
"""Trainium2 Bass kernel for the shared-weight transformer encoder with a
Conv1d-ensemble FFN (nn_MCAT_23630910062939).

Sharding: data-parallel over batch - each of the 8 NeuronCores computes one
full batch element; no collectives.  Residual stream lives on-chip in
feature-major layout x^T [D, S] (bf16); all matmuls run in fp8e4m3 with
MatmulPerfMode.DoubleRow (contraction pairs packed along a free dim), with
per-tensor power-of-two quantization scales folded into the surrounding
bias/scale ops.  Softmax exp is the ACT-engine critical path; the emission
order overlaps conv/LN/projection work (PE/DVE/Pool) under the exp stream,
including cross-layer overlap via query-half splitting.
"""
import sys
sys.path.insert(0, '/opt/trn_rl_repo')
import math
import numpy as np
import ml_dtypes

from contextlib import ExitStack
import concourse.bass as bass
import concourse.mybir as mybir
import concourse.tile as tile
from concourse import bacc, library_config
from concourse.bass_utils import run_bass_kernel_spmd
from concourse.dve_ops import RECIPROCAL_APPROX_NR

P = 128
D = 1024
S = 1024
H = 16
DK = 64
CH = 8            # 128-row feature chunks
CP = 4            # 256-row chunk pairs (DoubleRow)
NH = 2            # 512-col token halves
NL = 2
N_CORES = 8
EPS_BN = 1e-5
MAGIC = 0x5f3759df

f32 = mybir.dt.float32
i32 = mybir.dt.int32
bf16 = mybir.dt.bfloat16
fp8 = mybir.dt.float8e4
AF = mybir.ActivationFunctionType
OP = mybir.AluOpType
PM = mybir.MatmulPerfMode
E4 = ml_dtypes.float8_e4m3

# branch -> taps in cw pack order: (pack_idx, shift); each branch holds its
# hi taps then its lo (fp8 residual) taps, packed so 6-tap fetch groups are
# consumed in order (branch0: groups 0-1, branch1: 1-2, branch2: 2).
BRANCH_TAPS = [
    [(0, -2), (1, -1), (2, 0), (3, 1), (4, 2),
     (5, -2), (6, -1), (7, 0), (8, 1), (9, 2)],        # filter 5 hi+lo
    [(10, -1), (11, 0), (12, 1), (13, -1), (14, 0), (15, 1)],  # filter 3 hi+lo
    [(16, 0), (17, 0)],                                        # filter 1 hi+lo
]
N_TAPS = 18
NTG = 3  # 6-tap fetch groups


def _build(sc, debug=False):
    """sc: dict with quant scales kq, kk, kv, ko, kc (floats, powers of 2)."""
    nc = bacc.Bacc(None, target_bir_lowering=False)
    names = {}

    def reg(t, key):
        names[key] = t.name
        return t

    iq, ik, iv, io, ic_ = (1.0 / sc['kq'], 1.0 / sc['kk'], 1.0 / sc['kv'],
                           1.0 / sc['ko'], 1.0 / sc['kc'])

    with tile.TileContext(nc) as tc, ExitStack() as stack:
        with tc.tile_pool(name="dram", bufs=1, space="DRAM") as dram:
            xt_d = reg(dram.tile([D, S], bf16, kind="ExternalInput", name="xt"), "xt")
            wq_d = reg(dram.tile([P, CH, CP, 2, P], fp8, kind="ExternalInput", name="wq"), "wq")
            wk_d = reg(dram.tile([P, CH, CP, 2, P], fp8, kind="ExternalInput", name="wk"), "wk")
            wo_d = reg(dram.tile([P, CH, CP, 2, P], fp8, kind="ExternalInput", name="wo"), "wo")
            wv_d = reg(dram.tile([P, CP, 2, D], fp8, kind="ExternalInput", name="wv"), "wv")
            cw_d = reg(dram.tile([NTG, CH, P, 6, CP, 2, P], fp8, kind="ExternalInput", name="cw"), "cw")
            bq_d = reg(dram.tile([P, CH], f32, kind="ExternalInput", name="bq"), "bq")
            bk_d = reg(dram.tile([P, CH], f32, kind="ExternalInput", name="bk"), "bk")
            bo_d = reg(dram.tile([P, CH], f32, kind="ExternalInput", name="bo"), "bo")
            bv_d = reg(dram.tile([1, D], f32, kind="ExternalInput", name="bv"), "bv")
            cb_d = reg(dram.tile([P, 3, CH], f32, kind="ExternalInput", name="cb"), "cb")
            icc_d = reg(dram.tile([P, CH], f32, kind="ExternalInput", name="icc"), "icc")
            yt_d = reg(dram.tile([D, S], f32, kind="ExternalOutput", name="yt"), "yt")

            def dbg(key, ap, shape, dtype):
                if not debug:
                    return
                t = dram.tile(shape, dtype, kind="ExternalOutput", name=f"dbg_{key}")
                names[f"dbg_{key}"] = t.name
                nc.sync.dma_start(t[:], ap)

            nc.gpsimd.load_library(library_config.attn)

            glob = stack.enter_context(tc.tile_pool(name="glob", bufs=1))
            rot = stack.enter_context(tc.tile_pool(name="rot", bufs=1))
            psum = stack.enter_context(tc.tile_pool(name="psum", bufs=1, space="PSUM"))

            # ---------------- persistent tiles ----------------
            x = glob.tile([P, CH, S], bf16, tag="x")
            act1 = glob.tile([P, CH, S], fp8, tag="act1")
            act2 = glob.tile([P, CH, S + 4], fp8, tag="act2")
            qt = glob.tile([P, 4, 2, S], fp8, tag="qt")    # [32-group, h//4, dkslot, key/query tok]
            kt = glob.tile([P, 4, 2, S], fp8, tag="kt")
            ot = glob.tile([P, CH, S], fp8, tag="ot")
            wqr = glob.tile([P, CH, CP, 2, P], fp8, tag="wqr")
            wkr = glob.tile([P, CH, CP, 2, P], fp8, tag="wkr")
            wor = glob.tile([P, CH, CP, 2, P], fp8, tag="wor")
            vwr = glob.tile([P, CP, 2, D], fp8, tag="vwr")
            bq_sb = glob.tile([P, CH], f32, tag="bq")
            bk_sb = glob.tile([P, CH], f32, tag="bk")
            bo_sb = glob.tile([P, CH], f32, tag="bo")
            cb_sb = glob.tile([P, 3, CH], f32, tag="cb")
            icc_sb = glob.tile([P, CH], f32, tag="icc")
            bv_row = glob.tile([1, D], f32, tag="bvr")
            bv_b = glob.tile([P, D], f32, tag="bvb")
            ones_bf = glob.tile([P, 1], bf16, tag="ones")
            zeros = glob.tile([P, 512], bf16, tag="zeros")
            rowbuf0 = glob.tile([1, 512], f32, tag="rowbuf0")
            rowbuf1 = glob.tile([1, 512], f32, tag="rowbuf1")
            rowbuf2 = glob.tile([1, 512], f32, tag="rowbuf2")
            rowbuf3 = glob.tile([1, 512], f32, tag="rowbuf3")

            # ---------------- prelude ----------------
            nc.sync.dma_start(wqr[:], wq_d[:])
            nc.sync.dma_start(wkr[:], wk_d[:])
            nc.sync.dma_start(wor[:], wo_d[:])
            nc.sync.dma_start(vwr[:], wv_d[:])
            nc.sync.dma_start(bq_sb[:], bq_d[:])
            nc.sync.dma_start(bk_sb[:], bk_d[:])
            nc.sync.dma_start(bo_sb[:], bo_d[:])
            nc.sync.dma_start(cb_sb[:], cb_d[:])
            nc.sync.dma_start(icc_sb[:], icc_d[:])
            nc.sync.dma_start(bv_row[:], bv_d[:])
            for c in range(CH):
                nc.sync.dma_start(x[:, c, :], xt_d[c * P:(c + 1) * P, :])
            nc.gpsimd.partition_broadcast(bv_b[:], bv_row[:])
            nc.vector.memset(ones_bf[:], 1.0)
            nc.vector.memset(zeros[:], 0.0)
            nc.vector.memset(act2[:, :, 0:2], 0.0)
            nc.vector.memset(act2[:, :, S + 2:S + 4], 0.0)

            # vt: per layer (bufs=2) [P, tokchunk, h*65]; ones col set per alloc
            vt_of = {}

            def new_vt():
                t = rot.tile([P, CH, H * 80], fp8, tag="vt", bufs=2)
                v5 = t[:].rearrange("p c (h e) -> p c h e", e=80)
                nc.vector.memset(v5[:, :, :, 64:65], 1.0)
                nc.gpsimd.memset(v5[:, :, :, 65:80], 0.0)
                return t

            # ---------------- helpers ----------------
            def half_cols(n):
                return slice(n * 512, (n + 1) * 512)

            def ln_stats(n):
                """mean/msq chains for token-half n -> work-pool psum tile
                with sum(x) at row 0, sum(x^2) at row 32."""
                cols = half_cols(n)
                st = psum.tile([P, 512], f32, tag="work", bufs=3)
                for c in range(CH):
                    sq = rot.tile([P, 512], bf16, tag="sq", bufs=2)
                    nc.vector.tensor_tensor(sq[:], x[:, c, cols], x[:, c, cols], OP.mult)
                    nc.tensor.matmul(st[0:1, :], ones_bf[:], x[:, c, cols],
                                     start=(c == 0), stop=(c == CH - 1))
                    nc.tensor.matmul(st[32:33, :], ones_bf[:], sq[:],
                                     start=(c == 0), stop=(c == CH - 1))
                return st

            def ln_rows(st):
                """-> (inv_b, minv_b) [P,512] bf16 broadcasts from stats tile."""
                # four rotating row slots: r0 (t, y1), r1 (v1), r2 (h, z, z2),
                # r3 (s, inv_f)
                t = rowbuf0[:]
                nc.scalar.activation(t, st[0:1, :], AF.Square)
                v1 = rowbuf1[:]
                nc.vector.scalar_tensor_tensor(v1, t, -1.0 / D,
                                               st[32:33, :],
                                               op0=OP.mult, op1=OP.add)
                # rsqrt(v1/(D-1)) via bit-trick seed with the /(D-1) ~= 2^-10
                # folded into the magic (+5 exponent halves) and the NR scale
                h_ = rowbuf2[:]
                nc.vector.tensor_scalar(h_.bitcast(i32), v1.bitcast(i32), 1, None,
                                        op0=OP.logical_shift_right)
                s_ = rowbuf3[:]
                nc.vector.tensor_scalar(s_.bitcast(i32), h_.bitcast(i32), MAGIC + 5 * 0x800000, -1,
                                        op0=OP.subtract, op1=OP.mult)
                z_ = rowbuf2[:]
                nc.vector.scalar_tensor_tensor(z_, v1, 0.5 / (D - 1), s_, op0=OP.mult, op1=OP.mult)
                inv_f = rowbuf0[:]
                nc.vector._custom_dve(RECIPROCAL_APPROX_NR, out=inv_f, in0=z_, in1=s_, s0=1.5)
                inv_bf = rot.tile([1, 512], bf16, tag="invr", bufs=1)
                nc.vector.tensor_scalar(inv_bf[:], inv_f, 1.0, None, op0=OP.mult)
                minv_bf = rot.tile([1, 512], bf16, tag="minvr", bufs=1)
                nc.vector.scalar_tensor_tensor(minv_bf[:], st[0:1, :],
                                               1.0 / D, inv_f, op0=OP.mult, op1=OP.mult)
                inv_b = rot.tile([P, 512], bf16, tag="invb", bufs=1)
                minv_b = rot.tile([P, 512], bf16, tag="minvb", bufs=1)
                nc.gpsimd.partition_broadcast(inv_b[:], inv_bf[:])
                nc.gpsimd.partition_broadcast(minv_b[:], minv_bf[:])
                return inv_b, minv_b

            def ln_apply(n, inv_b, minv_b, dst, pad, edge=0):
                """normalize x half n -> dst (fp8, pad offset); if edge>0 the
                first `edge` cols are emitted separately first (to unblock the
                conv seam that reads only the B-half edge)."""
                cols = half_cols(n)
                if edge:
                    for c in range(CH):
                        te = rot.tile([P, 512], bf16, tag="lnt", bufs=1)
                        nc.vector.tensor_tensor(te[:, 0:edge], x[:, c, n * 512:n * 512 + edge],
                                                inv_b[:, 0:edge], OP.mult)
                        nc.gpsimd.tensor_tensor(dst[:, c, pad + n * 512:pad + n * 512 + edge],
                                                te[:, 0:edge], minv_b[:, 0:edge], OP.subtract)
                for c in range(CH):
                    t = rot.tile([P, 512], bf16, tag="lnt", bufs=1)
                    nc.vector.tensor_tensor(t[:, 0:512 - edge], x[:, c, n * 512 + edge:(n + 1) * 512],
                                            inv_b[:, edge:], OP.mult)
                    nc.gpsimd.tensor_tensor(dst[:, c, pad + n * 512 + edge:pad + (n + 1) * 512],
                                            t[:, 0:512 - edge], minv_b[:, edge:], OP.subtract)

            def lnf_apply(n, inv_b, minv_b):
                """final LN half n -> yt (f32) via DVE, DMA out."""
                cols = half_cols(n)
                for c in range(CH):
                    t = rot.tile([P, 512], bf16, tag="lnt", bufs=1)
                    nc.vector.tensor_tensor(t[:], x[:, c, cols], inv_b[:], OP.mult)
                    o = rot.tile([P, 512], f32, tag="yts", bufs=2)
                    nc.vector.tensor_tensor(o[:], t[:], minv_b[:], OP.subtract)
                    nc.sync.dma_start(yt_d[c * P:(c + 1) * P, cols], o[:])

            def qkproj(m, n, on_act=False):
                """Q and K projections for out-chunk m, token-half n; writes
                qt/kt via fp8 staging + partition-shuffle DMAs."""
                cols = half_cols(n)
                for (wres, bias, dstT, ksc) in ((wqr, bq_sb, qt, iq), (wkr, bk_sb, kt, ik)):
                    ps = psum.tile([P, 512], f32, tag="work", bufs=3)
                    for cp in range(CP):
                        nc.tensor.matmul(ps[:], wres[:, m, cp, :, :],
                                         act1[:, 2 * cp:2 * cp + 2, cols],
                                         start=(cp == 0), stop=(cp == CP - 1),
                                         perf_mode=PM.DoubleRow)
                    stg = rot.tile([P, 512], fp8, tag="stg", bufs=2)
                    if on_act:
                        nc.scalar.activation(stg[:], ps[:], AF.Identity,
                                             bias=bias[:, m:m + 1], scale=ksc)
                    else:
                        nc.vector.tensor_scalar(stg[:], ps[:], ksc, bias[:, m:m + 1],
                                                op0=OP.mult, op1=OP.add)
                    base = 64 * (m % 2)
                    nc.sync.dma_start(dstT[base:base + 64, m // 2, 0, cols], stg[0:64, :])
                    nc.sync.dma_start(dstT[base:base + 64, m // 2, 1, cols], stg[64:128, :])

            def vproj(vt, m, fh):
                """V projection token-chunk m, feature-half fh -> vt."""
                ps = psum.tile([P, 512], f32, tag="work", bufs=3)
                for cp in range(CP):
                    nc.tensor.matmul(ps[:], act1[:, 2 * cp:2 * cp + 2, m * P:(m + 1) * P],
                                     vwr[:, cp, :, 512 * fh:512 * (fh + 1)],
                                     start=(cp == 0), stop=(cp == CP - 1),
                                     perf_mode=PM.DoubleRow)
                v5 = vt[:].rearrange("p c (h e) -> p c h e", e=80)
                dst = v5[:, m, 8 * fh:8 * (fh + 1), 0:64]
                src = ps[:].rearrange("p (h e) -> p h e", e=64)
                bvv = bv_b[:, 512 * fh:512 * (fh + 1)].rearrange("p (h e) -> p h e", e=64)
                nc.vector.scalar_tensor_tensor(dst, src, iv, bvv, op0=OP.mult, op1=OP.add)

            def scores_exp(h, n, kclo, kchi, pexp_t):
                """score matmuls + exp for head h, query-half n, key chunks
                [kclo, kchi); exp into half-width pexp_t[:, kc-kclo, :] (fp8)."""
                g = h % 4
                hs = h // 4
                qcols = half_cols(n)
                for kc2 in range(kclo, kchi, 2):
                    scps = psum.tile([P, 2, 512], f32, tag="scps", bufs=2)
                    for j in range(2):
                        kc = kc2 + j
                        nc.tensor.matmul(scps[:, j, :],
                                         kt[32 * g:32 * g + 32, hs, :, kc * P:(kc + 1) * P],
                                         qt[32 * g:32 * g + 32, hs, :, qcols],
                                         start=True, stop=True, perf_mode=PM.DoubleRow,
                                         tile_position=(32 * g, 0))
                    ko_ = kc2 - kclo
                    nc.scalar.activation(pexp_t[:, ko_:ko_ + 2, :], scps[:], AF.Exp)

            def av_norm(vt, h, n, pexp_lo, pexp_hi):
                """AV + denominator + normalize -> ot for head h, q-half n."""
                cols = half_cols(n)
                av = psum.tile([P, 512], f32, tag="misc", bufs=1)
                for cp in range(CP):
                    pt = pexp_lo if cp < 2 else pexp_hi
                    po = 2 * cp if cp < 2 else 2 * (cp - 2)
                    nc.tensor.matmul(av[0:80, :],
                                     vt[:, 2 * cp:2 * cp + 2, 80 * h:80 * h + 80],
                                     pt[:, po:po + 2, :],
                                     start=(cp == 0), stop=(cp == CP - 1),
                                     perf_mode=PM.DoubleRow)
                rrow = rot.tile([1, 512], f32, tag="rrow", bufs=1)
                nc.vector.reciprocal(rrow[:], av[64:65, :])
                rb = rot.tile([64, 512], f32, tag="rb", bufs=1)
                nc.gpsimd.partition_broadcast(rb[:], rrow[:])
                off = 64 * (h % 2)
                if debug and h == 0 and n == 0:
                    avc = rot.tile([P, 512], f32, tag="yts", bufs=2)
                    nc.vector.tensor_copy(avc[0:65, :], av[0:65, :])
                    dbg("av0", avc[:], [P, 512], f32)
                    dbg("rrow0", rrow[:], [1, 512], f32)
                nc.vector.tensor_tensor(ot[off:off + 64, h // 2, cols],
                                        av[0:64, :], rb[:], OP.mult)

            def wo_m(m, n):
                """output projection chunk m, token-half n + residual add."""
                cols = half_cols(n)
                ps = psum.tile([P, 512], f32, tag="work", bufs=3)
                for cp in range(CP):
                    nc.tensor.matmul(ps[:], wor[:, m, cp, :, :], ot[:, 2 * cp:2 * cp + 2, cols],
                                     start=(cp == 0), stop=(cp == CP - 1),
                                     perf_mode=PM.DoubleRow)
                tmp = rot.tile([P, 512], bf16, tag="wotmp", bufs=2)
                nc.scalar.activation(tmp[:], ps[:], AF.Identity,
                                     bias=bo_sb[:, m:m + 1], scale=io)
                nc.vector.tensor_tensor(x[:, m, cols], tmp[:], x[:, m, cols], OP.add)

            def conv_m(m, lo, hi, relu_act=False):
                """conv ensemble for out-chunk m, output cols [lo, hi)."""
                w = hi - lo
                scs = []
                grp = {}
                for bi, taps in enumerate(BRANCH_TAPS):
                    cps = psum.tile([P, 512], f32, tag="work", bufs=3)
                    first = True
                    for (tp, shift) in taps:
                        gi, tl = tp // 6, tp % 6
                        if gi not in grp:
                            wg = rot.tile([P, 6, CP, 2, P], fp8, tag="wcv", bufs=2, name="wg")
                            nc.sync.dma_start(wg[:], cw_d[gi, m])
                            grp[gi] = wg
                        u0 = 2 + shift + lo
                        for cp in range(CP):
                            nc.tensor.matmul(cps[:, 0:w], grp[gi][:, tl, cp, :, :],
                                             act2[:, 2 * cp:2 * cp + 2, u0:u0 + w],
                                             start=first, stop=(cp == CP - 1 and tp == taps[-1][0]),
                                             perf_mode=PM.DoubleRow)
                            first = False
                    sc_t = rot.tile([P, 512], bf16, tag=f"sc{bi}", bufs=2)
                    if relu_act:
                        nc.scalar.activation(sc_t[:, 0:w], cps[:, 0:w], AF.Relu,
                                             bias=cb_sb[:, bi, m:m + 1], scale=1.0)
                    else:
                        nc.vector.scalar_tensor_tensor(sc_t[:, 0:w], cps[:, 0:w], cb_sb[:, bi, m:m + 1],
                                                       zeros[:, 0:w], op0=OP.add, op1=OP.max)
                    scs.append(sc_t)
                t1 = rot.tile([P, 512], bf16, tag="cmb1", bufs=1)
                nc.gpsimd.tensor_tensor(t1[:, 0:w], scs[0][:, 0:w], scs[1][:, 0:w], OP.add)
                t2 = rot.tile([P, 512], bf16, tag="cmb2", bufs=1)
                nc.gpsimd.tensor_tensor(t2[:, 0:w], t1[:, 0:w], scs[2][:, 0:w], OP.add)
                nc.vector.scalar_tensor_tensor(x[:, m, lo:hi], t2[:, 0:w], icc_sb[:, m:m + 1],
                                               x[:, m, lo:hi], op0=OP.mult, op1=OP.add)

            # ---------------- PSUM layout ----------------

            # ---------------- emission ----------------
            # Interleaved schedule: while ACT streams exp for one query-half,
            # PE runs the other half's AV/conv/LN/projections and the next
            # layer's lead-in.  pexp is split into lo/hi key-half tiles;
            # pxAlo has bufs=H because the next layer's lo-exps (p1) stay
            # live until the AVs in the p2 loop.
            pex = {}

            def new_px(key, tag, bufs):
                t = rot.tile([P, CH // 2, 512], fp8, tag=tag, bufs=bufs, name="px")
                pex[key] = t
                return t

            # ---- layer-0 lead-in ----
            stA = ln_stats(0)
            stB = ln_stats(1)
            iA, mA = ln_rows(stA)
            ln_apply(0, iA, mA, act1, 0)
            iB, mB = ln_rows(stB)
            ln_apply(1, iB, mB, act1, 0)
            dbg("act1", act1[:], [P, CH, S], fp8)
            dbg("ivA", iA[:], [P, 512], bf16)
            dbg("mvA", mA[:], [P, 512], bf16)
            for m in range(CH):
                qkproj(m, 0, on_act=True)
                qkproj(m, 1, on_act=True)
            dbg("qt", qt[:], [P, 4, 2, S], fp8)
            dbg("kt", kt[:], [P, 4, 2, S], fp8)
            vt_of[0] = new_vt()
            for m in range(CH):
                for fh in range(2):
                    vproj(vt_of[0], m, fh)
            dbg("vt", vt_of[0][:], [P, CH, H * 80], fp8)
            # attn A of layer 0, AV lag-2
            for h in range(H):
                scores_exp(h, 0, 0, 4, new_px((0, 0, h, 0), "pxAlo", H))
                scores_exp(h, 0, 4, 8, new_px((0, 0, h, 1), "pxAhi", 2))
                if h == 0:
                    dbg("pexlo0", pex[(0, 0, 0, 0)][:], [P, CH // 2, 512], fp8)
                    dbg("pexhi0", pex[(0, 0, 0, 1)][:], [P, CH // 2, 512], fp8)
                if h >= 1:
                    av_norm(vt_of[0], h - 1, 0, pex[(0, 0, h - 1, 0)], pex[(0, 0, h - 1, 1)])
            av_norm(vt_of[0], H - 1, 0, pex[(0, 0, H - 1, 0)], pex[(0, 0, H - 1, 1)])
            dbg("otA", ot[:], [P, CH, S], fp8)
            for m in range(CH):
                wo_m(m, 0)
            dbg("xA", x[:], [P, CH, S], bf16)
            st2A = ln_stats(0)
            i2A, m2A = ln_rows(st2A)
            ln_apply(0, i2A, m2A, act2, 2)

            for l in range(NL):
                last = (l == NL - 1)
                # ---- B loop: scores/exp B + AV-B (lag-1) + conv-A cols [0:510)
                for h in range(H):
                    scores_exp(h, 1, 0, 4, new_px((l, 1, h, 0), "pxBlo", 2))
                    scores_exp(h, 1, 4, 8, new_px((l, 1, h, 1), "pxBhi", 2))
                    if h >= 1:
                        av_norm(vt_of[l], h - 1, 1, pex[(l, 1, h - 1, 0)], pex[(l, 1, h - 1, 1)])
                    if 1 <= h <= 8:
                        conv_m(h - 1, 0, 480)
                av_norm(vt_of[l], H - 1, 1, pex[(l, 1, H - 1, 0)], pex[(l, 1, H - 1, 1)])
                for m in range(CH):
                    wo_m(m, 1)
                st2B = ln_stats(1)
                i2B, m2B = ln_rows(st2B)
                ln_apply(1, i2B, m2B, act2, 2, edge=32)

                if not last:
                    # ---- A' p1 loop: conv seam + chunk2, LN1'A, proj'A,
                    # scores'/exp' keys 0:4 (all 16 pexp-lo tiles stay live)
                    vt_of[l + 1] = new_vt()
                    for h in range(H):
                        if h == 0:
                            for mm_ in range(CH):
                                conv_m(mm_, 480, 512)      # seam cols (needs act2 B edge)
                            st1A = ln_stats(0)
                            i1A, m1A = ln_rows(st1A)
                            ln_apply(0, i1A, m1A, act1, 0)
                        if h < CH:
                            qkproj(h, 0, on_act=True)
                        # scores for head h need qt'/kt' chunk m=h//2 <= h (emitted)
                        scores_exp(h, 0, 0, 4, new_px((l + 1, 0, h, 0), "pxAlo", H))
                        if 2 <= h <= 9:
                            conv_m(h - 2, 512, S)          # cols [512:1024)

                    # ---- A' p2 loop: LN1'B + proj'B + vproj'4-7 + keys 4:8 + AV'
                    for h in range(H):
                        if h == 0:
                            st1B = ln_stats(1)
                            i1B, m1B = ln_rows(st1B)
                            ln_apply(1, i1B, m1B, act1, 0)
                        if h < CH:
                            qkproj(h, 1, on_act=True)
                        if h == 0:
                            for mm_ in range(CH):
                                vproj(vt_of[l + 1], mm_, 0)
                                vproj(vt_of[l + 1], mm_, 1)
                        scores_exp(h, 0, 4, 8, new_px((l + 1, 0, h, 1), "pxAhi", 2))
                        if h >= 1:
                            av_norm(vt_of[l + 1], h - 1, 0,
                                    pex[(l + 1, 0, h - 1, 0)], pex[(l + 1, 0, h - 1, 1)])
                    av_norm(vt_of[l + 1], H - 1, 0, pex[(l + 1, 0, H - 1, 0)], pex[(l + 1, 0, H - 1, 1)])
                    for m in range(CH):
                        wo_m(m, 0)
                    st2A_ = ln_stats(0)
                    i2A_, m2A_ = ln_rows(st2A_)
                    ln_apply(0, i2A_, m2A_, act2, 2)
                else:
                    # ---- final: conv seam + chunk2 + final LN (A overlapped)
                    for m in range(CH):
                        conv_m(m, 480, 512, relu_act=True)
                    stfA = ln_stats(0)
                    for m in range(CH):
                        conv_m(m, 512, S, relu_act=True)
                        if m == 0:
                            ifA, mfA = ln_rows(stfA)
                            lnf_apply(0, ifA, mfA)
                    stfB = ln_stats(1)
                    ifB, mfB = ln_rows(stfB)
                    lnf_apply(1, ifB, mfB)

    nc.compile()
    return nc, names


_BUILT = None


def _pow2_scale(arr, target=192.0):
    am = float(np.abs(arr).max())
    if am <= 0:
        return 1.0
    return 2.0 ** math.floor(math.log2(target / am))


def _pack_dr(W, k):
    """W [D_in, D_out] (already folded+scaled by caller except k) ->
    [P, CH_m, CP, 2, P] fp8 with element [p, m, cp, i, j] = k*W[256cp+128i+p, 128m+j]."""
    Wq = (W * k).astype(E4)
    return np.ascontiguousarray(
        Wq.reshape(CP, 2, P, CH, P).transpose(2, 3, 0, 1, 4))


# local column permutation inside each 128-out-chunk for Q/K so that the
# staging rows land shuffle-DMA-contiguous: [a_dk0:32, b_dk0:32, a_dk32:64, b_dk32:64]
_QK_PERM = np.concatenate([np.arange(0, 32), np.arange(64, 96),
                           np.arange(32, 64), np.arange(96, 128)])


def _perm_qk_cols(W):
    Wc = W.reshape(D, CH, P)[:, :, _QK_PERM]
    return np.ascontiguousarray(Wc.reshape(D, D))


def _perm_qk_bias(b):
    return np.ascontiguousarray(b.reshape(CH, P)[:, _QK_PERM].reshape(D))


def _pack_bias(b):
    return np.ascontiguousarray(b.reshape(CH, P).T)


def _prep(inputs):
    f = lambda kk: np.asarray(inputs[kk], np.float32)
    a1, b1 = f('ln1_a'), f('ln1_b')
    a2, b2 = f('ln2_a'), f('ln2_b')
    wq, wk, wv, wo = f('wq'), f('wk'), f('wv'), f('wo')
    bq, bk, bv, bo = f('bq'), f('bk'), f('bv'), f('bo')

    wq_e = _perm_qk_cols(a1[:, None] * wq / 8.0)
    bq_e = _perm_qk_bias((bq + b1 @ wq) / 8.0)
    wk_e = _perm_qk_cols(a1[:, None] * wk)
    bk_e = _perm_qk_bias(bk + b1 @ wk)
    wv_e = a1[:, None] * wv
    bv_e = bv + b1 @ wv

    sc = {'kq': _pow2_scale(wq_e), 'kk': _pow2_scale(wk_e),
          'kv': _pow2_scale(wv_e), 'ko': _pow2_scale(wo)}

    d = {}
    d['wq'] = _pack_dr(wq_e, sc['kq'])
    d['bq'] = _pack_bias(bq_e)
    d['wk'] = _pack_dr(wk_e, sc['kk'])
    d['bk'] = _pack_bias(bk_e)
    # V: moving-side pack [P, CP, 2, D]: [p, cp, i, f] = kv*Wv[256cp+128i+p, f]
    d['wv'] = np.ascontiguousarray(
        (wv_e * sc['kv']).astype(E4).reshape(CP, 2, P, D).transpose(2, 0, 1, 3))
    d['bv'] = bv_e.reshape(1, D)
    d['wo'] = _pack_dr(wo, sc['ko'])
    d['bo'] = _pack_bias(bo)

    # conv: fold BN + ensemble mean + ln2 affine
    Wf_all, bias_all = [], []
    for bi, fs in enumerate((5, 3, 1)):
        i = 3 - bi
        W = f(f'conv_w{i}')
        b = f(f'conv_b{i}')
        g, beta = f(f'bn_g{i}'), f(f'bn_b{i}')
        m, v = f(f'bn_m{i}'), f(f'bn_v{i}')
        s = g / np.sqrt(v + EPS_BN)
        Wf_all.append(W * s[:, None, None] * a2[None, :, None] / 3.0)
        bias_all.append((b + W.sum(axis=2) @ b2 - m) * s + beta)
    bias_all = [bb / 3.0 for bb in bias_all]
    sc['kc'] = 1.0  # per-channel scales via icc instead
    # per-output-channel pow2 scale over all branches/taps
    am = np.max(np.stack([np.abs(Wf).max(axis=(1, 2)) for Wf in Wf_all]), axis=0)
    kvec = 2.0 ** np.floor(np.log2(192.0 / np.maximum(am, 1e-20)))

    cw = np.empty((NTG, CH, P, 6, CP, 2, P), E4)
    cb = np.empty((P, 3, CH), np.float32)
    for bi in range(3):
        cb[:, bi, :] = _pack_bias(bias_all[bi] * kvec)
        ntap = len(BRANCH_TAPS[bi]) // 2
        for j in range(ntap):
            tp_hi = BRANCH_TAPS[bi][j][0]
            tp_lo = BRANCH_TAPS[bi][ntap + j][0]
            Wk = np.ascontiguousarray(Wf_all[bi][:, :, j].T) * kvec[None, :]
            hi8 = Wk.astype(E4)
            resid = Wk - hi8.astype(np.float32)
            cw[tp_hi // 6, :, :, tp_hi % 6] = _pack_dr(hi8.astype(np.float32), 1.0).transpose(1, 0, 2, 3, 4)
            cw[tp_lo // 6, :, :, tp_lo % 6] = _pack_dr(resid, 1.0).transpose(1, 0, 2, 3, 4)
    d['cw'] = cw
    d['cb'] = cb
    d['icc'] = _pack_bias(1.0 / kvec)
    return d, sc


def kernel(**inputs):
    global _BUILT
    shared, sc = _prep(inputs)
    if _BUILT is None:
        _BUILT = _build(sc)
    nc, names = _BUILT
    x = np.asarray(inputs['x'], np.float32)
    in_maps = []
    for b in range(N_CORES):
        m = {names[k]: v for k, v in shared.items()}
        m[names['xt']] = np.ascontiguousarray(x[b].T).astype(ml_dtypes.bfloat16)
        in_maps.append(m)
    res = run_bass_kernel_spmd(nc, in_maps, core_ids=list(range(N_CORES)))
    af = np.asarray(inputs['lnf_a'], np.float32)
    bf = np.asarray(inputs['lnf_b'], np.float32)
    out = np.empty((N_CORES, S, D), np.float32)
    for b in range(N_CORES):
        yt = res.results[b][names['yt']]
        out[b] = yt.T * af[None, :] + bf[None, :]
    return out


# revision 41
# speedup vs baseline: 1.2207x; 1.0179x over previous
"""Trainium2 Bass kernel for the shared-weight transformer encoder with a
Conv1d-ensemble FFN (nn_MCAT_23630910062939).

Sharding: data-parallel over batch - each of the 8 NeuronCores computes one
full batch element; no collectives.  Residual stream lives on-chip in
feature-major layout x^T [D, S] (bf16); all matmuls run in fp8e4m3 with
MatmulPerfMode.DoubleRow (contraction pairs packed along a free dim), with
per-tensor power-of-two quantization scales folded into the surrounding
bias/scale ops.  Softmax exp is the ACT-engine critical path; the emission
order overlaps conv/LN/projection work (PE/DVE/Pool) under the exp stream,
including cross-layer overlap via query-half splitting.
"""
import sys
sys.path.insert(0, '/opt/trn_rl_repo')
import math
import numpy as np
import ml_dtypes

from contextlib import ExitStack
import concourse.bass as bass
import concourse.mybir as mybir
import concourse.tile as tile
from concourse import bacc, library_config
from concourse.bass_utils import run_bass_kernel_spmd
from concourse.dve_ops import RECIPROCAL_APPROX_NR

P = 128
D = 1024
S = 1024
H = 16
DK = 64
CH = 8            # 128-row feature chunks
CP = 4            # 256-row chunk pairs (DoubleRow)
NH = 2            # 512-col token halves
NL = 2
N_CORES = 8
EPS_BN = 1e-5
MAGIC = 0x5f3759df

f32 = mybir.dt.float32
i32 = mybir.dt.int32
bf16 = mybir.dt.bfloat16
fp8 = mybir.dt.float8e4
AF = mybir.ActivationFunctionType
OP = mybir.AluOpType
PM = mybir.MatmulPerfMode
E4 = ml_dtypes.float8_e4m3

# branch -> taps in cw pack order: (pack_idx, shift); each branch holds its
# hi taps then its lo (fp8 residual) taps, packed so 6-tap fetch groups are
# consumed in order (branch0: groups 0-1, branch1: 1-2, branch2: 2).
BRANCH_TAPS = [
    [(0, -2), (1, -1), (2, 0), (3, 1), (4, 2),
     (5, -2), (6, -1), (7, 0), (8, 1), (9, 2)],        # filter 5 hi+lo
    [(10, -1), (11, 0), (12, 1), (13, -1), (14, 0), (15, 1)],  # filter 3 hi+lo
    [(16, 0), (17, 0)],                                        # filter 1 hi+lo
]
N_TAPS = 18
NTG = 3  # 6-tap fetch groups


def _build(sc, debug=False):
    """sc: dict with quant scales kq, kk, kv, ko, kc (floats, powers of 2)."""
    nc = bacc.Bacc(None, target_bir_lowering=False)
    names = {}

    def reg(t, key):
        names[key] = t.name
        return t

    iq, ik, iv, io, ic_ = (1.0 / sc['kq'], 1.0 / sc['kk'], 1.0 / sc['kv'],
                           1.0 / sc['ko'], 1.0 / sc['kc'])

    with tile.TileContext(nc) as tc, ExitStack() as stack:
        with tc.tile_pool(name="dram", bufs=1, space="DRAM") as dram:
            xt_d = reg(dram.tile([D, S], bf16, kind="ExternalInput", name="xt"), "xt")
            wq_d = reg(dram.tile([P, CH, CP, 2, P], fp8, kind="ExternalInput", name="wq"), "wq")
            wk_d = reg(dram.tile([P, CH, CP, 2, P], fp8, kind="ExternalInput", name="wk"), "wk")
            wo_d = reg(dram.tile([P, CH, CP, 2, P], fp8, kind="ExternalInput", name="wo"), "wo")
            wv_d = reg(dram.tile([P, CP, 2, D], fp8, kind="ExternalInput", name="wv"), "wv")
            cw_d = reg(dram.tile([NTG, CH, P, 6, CP, 2, P], fp8, kind="ExternalInput", name="cw"), "cw")
            bq_d = reg(dram.tile([P, CH], f32, kind="ExternalInput", name="bq"), "bq")
            bk_d = reg(dram.tile([P, CH], f32, kind="ExternalInput", name="bk"), "bk")
            bo_d = reg(dram.tile([P, CH], f32, kind="ExternalInput", name="bo"), "bo")
            bv_d = reg(dram.tile([1, D], f32, kind="ExternalInput", name="bv"), "bv")
            cb_d = reg(dram.tile([P, 3, CH], f32, kind="ExternalInput", name="cb"), "cb")
            icc_d = reg(dram.tile([P, CH], f32, kind="ExternalInput", name="icc"), "icc")
            yt_d = reg(dram.tile([D, S], f32, kind="ExternalOutput", name="yt"), "yt")

            def dbg(key, ap, shape, dtype):
                if not debug:
                    return
                t = dram.tile(shape, dtype, kind="ExternalOutput", name=f"dbg_{key}")
                names[f"dbg_{key}"] = t.name
                nc.sync.dma_start(t[:], ap)

            nc.gpsimd.load_library(library_config.attn)

            glob = stack.enter_context(tc.tile_pool(name="glob", bufs=1))
            rot = stack.enter_context(tc.tile_pool(name="rot", bufs=1))
            psum = stack.enter_context(tc.tile_pool(name="psum", bufs=1, space="PSUM"))

            # ---------------- persistent tiles ----------------
            x = glob.tile([P, CH, S], bf16, tag="x")
            act1 = glob.tile([P, CH, S], fp8, tag="act1")
            act2 = glob.tile([P, CH, S + 4], fp8, tag="act2")
            qt = glob.tile([P, 4, 2, S], fp8, tag="qt")    # [32-group, h//4, dkslot, key/query tok]
            kt = glob.tile([P, 4, 2, S], fp8, tag="kt")
            ot = glob.tile([P, CH, S], fp8, tag="ot")
            wqr = glob.tile([P, CH, CP, 2, P], fp8, tag="wqr")
            wkr = glob.tile([P, CH, CP, 2, P], fp8, tag="wkr")
            wor = glob.tile([P, CH, CP, 2, P], fp8, tag="wor")
            vwr = glob.tile([P, CP, 2, D], fp8, tag="vwr")
            bq_sb = glob.tile([P, CH], f32, tag="bq")
            bk_sb = glob.tile([P, CH], f32, tag="bk")
            bo_sb = glob.tile([P, CH], f32, tag="bo")
            cb_sb = glob.tile([P, 3, CH], f32, tag="cb")
            icc_sb = glob.tile([P, CH], f32, tag="icc")
            bv_row = glob.tile([1, D], f32, tag="bvr")
            bv_b = glob.tile([P, D], f32, tag="bvb")
            ones_bf = glob.tile([P, 1], bf16, tag="ones")
            zeros = glob.tile([P, 512], bf16, tag="zeros")
            rowbuf0 = glob.tile([1, 512], f32, tag="rowbuf0")
            rowbuf1 = glob.tile([1, 512], f32, tag="rowbuf1")
            rowbuf2 = glob.tile([1, 512], f32, tag="rowbuf2")
            rowbuf3 = glob.tile([1, 512], f32, tag="rowbuf3")

            # ---------------- prelude ----------------
            nc.sync.dma_start(wqr[:], wq_d[:])
            nc.sync.dma_start(wkr[:], wk_d[:])
            nc.sync.dma_start(wor[:], wo_d[:])
            nc.sync.dma_start(vwr[:], wv_d[:])
            nc.sync.dma_start(bq_sb[:], bq_d[:])
            nc.sync.dma_start(bk_sb[:], bk_d[:])
            nc.sync.dma_start(bo_sb[:], bo_d[:])
            nc.sync.dma_start(cb_sb[:], cb_d[:])
            nc.sync.dma_start(icc_sb[:], icc_d[:])
            nc.sync.dma_start(bv_row[:], bv_d[:])
            for c in range(CH):
                nc.sync.dma_start(x[:, c, :], xt_d[c * P:(c + 1) * P, :])
            nc.gpsimd.partition_broadcast(bv_b[:], bv_row[:])
            nc.vector.memset(ones_bf[:], 1.0)
            nc.vector.memset(zeros[:], 0.0)
            nc.vector.memset(act2[:, :, 0:2], 0.0)
            nc.vector.memset(act2[:, :, S + 2:S + 4], 0.0)

            # vt: per layer (bufs=2) [P, tokchunk, h*65]; ones col set per alloc
            vt_of = {}

            def new_vt():
                t = rot.tile([P, CH, H * 80], fp8, tag="vt", bufs=2)
                v5 = t[:].rearrange("p c (h e) -> p c h e", e=80)
                nc.vector.memset(v5[:, :, :, 64:65], 1.0)
                nc.gpsimd.memset(v5[:, :, :, 65:80], 0.0)
                return t

            # ---------------- helpers ----------------
            def half_cols(n):
                return slice(n * 512, (n + 1) * 512)

            def ln_stats(n):
                """mean/msq chains for token-half n -> work-pool psum tile
                with sum(x) at row 0, sum(x^2) at row 32."""
                cols = half_cols(n)
                st = psum.tile([P, 512], f32, tag="work", bufs=3)
                for c in range(CH):
                    sq = rot.tile([P, 512], bf16, tag="sq", bufs=2)
                    nc.vector.tensor_tensor(sq[:], x[:, c, cols], x[:, c, cols], OP.mult)
                    nc.tensor.matmul(st[0:1, :], ones_bf[:], x[:, c, cols],
                                     start=(c == 0), stop=(c == CH - 1))
                    nc.tensor.matmul(st[32:33, :], ones_bf[:], sq[:],
                                     start=(c == 0), stop=(c == CH - 1))
                return st

            def ln_rows(st):
                """-> (inv_b, minv_b) [P,512] bf16 broadcasts from stats tile."""
                # four rotating row slots: r0 (t, y1), r1 (v1), r2 (h, z, z2),
                # r3 (s, inv_f)
                t = rowbuf0[:]
                nc.scalar.activation(t, st[0:1, :], AF.Square)
                v1 = rowbuf1[:]
                nc.vector.scalar_tensor_tensor(v1, t, -1.0 / D,
                                               st[32:33, :],
                                               op0=OP.mult, op1=OP.add)
                # rsqrt(v1/(D-1)) via bit-trick seed with the /(D-1) ~= 2^-10
                # folded into the magic (+5 exponent halves) and the NR scale
                h_ = rowbuf2[:]
                nc.vector.tensor_scalar(h_.bitcast(i32), v1.bitcast(i32), 1, None,
                                        op0=OP.logical_shift_right)
                s_ = rowbuf3[:]
                nc.vector.tensor_scalar(s_.bitcast(i32), h_.bitcast(i32), MAGIC + 5 * 0x800000, -1,
                                        op0=OP.subtract, op1=OP.mult)
                z_ = rowbuf2[:]
                nc.vector.scalar_tensor_tensor(z_, v1, 0.5 / (D - 1), s_, op0=OP.mult, op1=OP.mult)
                inv_f = rowbuf0[:]
                nc.vector._custom_dve(RECIPROCAL_APPROX_NR, out=inv_f, in0=z_, in1=s_, s0=1.5)
                inv_bf = rot.tile([1, 512], bf16, tag="invr", bufs=1)
                nc.vector.tensor_scalar(inv_bf[:], inv_f, 1.0, None, op0=OP.mult)
                minv_bf = rot.tile([1, 512], bf16, tag="minvr", bufs=1)
                nc.vector.scalar_tensor_tensor(minv_bf[:], st[0:1, :],
                                               1.0 / D, inv_f, op0=OP.mult, op1=OP.mult)
                inv_b = rot.tile([P, 512], bf16, tag="invb", bufs=1)
                minv_b = rot.tile([P, 512], bf16, tag="minvb", bufs=1)
                nc.gpsimd.partition_broadcast(inv_b[:], inv_bf[:])
                nc.gpsimd.partition_broadcast(minv_b[:], minv_bf[:])
                return inv_b, minv_b

            def ln_apply(n, inv_b, minv_b, dst, pad, edge=0):
                """normalize x half n -> dst (fp8, pad offset); if edge>0 the
                first `edge` cols are emitted separately first (to unblock the
                conv seam that reads only the B-half edge)."""
                cols = half_cols(n)
                if edge:
                    for c in range(CH):
                        te = rot.tile([P, 512], bf16, tag="lnt", bufs=1)
                        nc.vector.tensor_tensor(te[:, 0:edge], x[:, c, n * 512:n * 512 + edge],
                                                inv_b[:, 0:edge], OP.mult)
                        nc.gpsimd.tensor_tensor(dst[:, c, pad + n * 512:pad + n * 512 + edge],
                                                te[:, 0:edge], minv_b[:, 0:edge], OP.subtract)
                for c in range(CH):
                    t = rot.tile([P, 512], bf16, tag="lnt", bufs=1)
                    nc.vector.tensor_tensor(t[:, 0:512 - edge], x[:, c, n * 512 + edge:(n + 1) * 512],
                                            inv_b[:, edge:], OP.mult)
                    nc.gpsimd.tensor_tensor(dst[:, c, pad + n * 512 + edge:pad + (n + 1) * 512],
                                            t[:, 0:512 - edge], minv_b[:, edge:], OP.subtract)

            def lnf_apply(n, inv_b, minv_b):
                """final LN half n -> yt (f32) via DVE, DMA out."""
                cols = half_cols(n)
                for c in range(CH):
                    t = rot.tile([P, 512], bf16, tag="lnt", bufs=1)
                    nc.vector.tensor_tensor(t[:], x[:, c, cols], inv_b[:], OP.mult)
                    o = rot.tile([P, 512], f32, tag="yts", bufs=2)
                    nc.vector.tensor_tensor(o[:], t[:], minv_b[:], OP.subtract)
                    nc.sync.dma_start(yt_d[c * P:(c + 1) * P, cols], o[:])

            def qkproj(m, n, on_act=False):
                """Q and K projections for out-chunk m, token-half n; writes
                qt/kt via fp8 staging + partition-shuffle DMAs."""
                cols = half_cols(n)
                for (wres, bias, dstT, ksc) in ((wqr, bq_sb, qt, iq), (wkr, bk_sb, kt, ik)):
                    ps = psum.tile([P, 512], f32, tag="work", bufs=3)
                    for cp in range(CP):
                        nc.tensor.matmul(ps[:], wres[:, m, cp, :, :],
                                         act1[:, 2 * cp:2 * cp + 2, cols],
                                         start=(cp == 0), stop=(cp == CP - 1),
                                         perf_mode=PM.DoubleRow)
                    stg = rot.tile([P, 512], fp8, tag="stg", bufs=2)
                    if on_act:
                        nc.scalar.activation(stg[:], ps[:], AF.Identity,
                                             bias=bias[:, m:m + 1], scale=ksc)
                    else:
                        nc.vector.tensor_scalar(stg[:], ps[:], ksc, bias[:, m:m + 1],
                                                op0=OP.mult, op1=OP.add)
                    base = 64 * (m % 2)
                    nc.sync.dma_start(dstT[base:base + 64, m // 2, 0, cols], stg[0:64, :])
                    nc.sync.dma_start(dstT[base:base + 64, m // 2, 1, cols], stg[64:128, :])

            def vproj(vt, m, fh):
                """V projection token-chunk m, feature-half fh -> vt."""
                ps = psum.tile([P, 512], f32, tag="work", bufs=3)
                for cp in range(CP):
                    nc.tensor.matmul(ps[:], act1[:, 2 * cp:2 * cp + 2, m * P:(m + 1) * P],
                                     vwr[:, cp, :, 512 * fh:512 * (fh + 1)],
                                     start=(cp == 0), stop=(cp == CP - 1),
                                     perf_mode=PM.DoubleRow)
                v5 = vt[:].rearrange("p c (h e) -> p c h e", e=80)
                dst = v5[:, m, 8 * fh:8 * (fh + 1), 0:64]
                src = ps[:].rearrange("p (h e) -> p h e", e=64)
                bvv = bv_b[:, 512 * fh:512 * (fh + 1)].rearrange("p (h e) -> p h e", e=64)
                nc.vector.scalar_tensor_tensor(dst, src, iv, bvv, op0=OP.mult, op1=OP.add)

            def scores_exp(h, n, kclo, kchi, pexp_t):
                """score matmuls + exp for head h, query-half n, key chunks
                [kclo, kchi); exp into half-width pexp_t[:, kc-kclo, :] (fp8)."""
                g = h % 4
                hs = h // 4
                qcols = half_cols(n)
                for kc2 in range(kclo, kchi, 2):
                    scps = psum.tile([P, 2, 512], f32, tag="scps", bufs=2)
                    for j in range(2):
                        kc = kc2 + j
                        nc.tensor.matmul(scps[:, j, :],
                                         kt[32 * g:32 * g + 32, hs, :, kc * P:(kc + 1) * P],
                                         qt[32 * g:32 * g + 32, hs, :, qcols],
                                         start=True, stop=True, perf_mode=PM.DoubleRow,
                                         tile_position=(32 * g, 0))
                    ko_ = kc2 - kclo
                    nc.scalar.activation(pexp_t[:, ko_:ko_ + 2, :], scps[:], AF.Exp)

            def av_norm(vt, h, n, pexp_lo, pexp_hi):
                """AV + denominator + normalize -> ot for head h, q-half n."""
                cols = half_cols(n)
                av = psum.tile([P, 512], f32, tag="misc", bufs=1)
                for cp in range(CP):
                    pt = pexp_lo if cp < 2 else pexp_hi
                    po = 2 * cp if cp < 2 else 2 * (cp - 2)
                    nc.tensor.matmul(av[0:80, :],
                                     vt[:, 2 * cp:2 * cp + 2, 80 * h:80 * h + 80],
                                     pt[:, po:po + 2, :],
                                     start=(cp == 0), stop=(cp == CP - 1),
                                     perf_mode=PM.DoubleRow)
                rrow = rot.tile([1, 512], f32, tag="rrow", bufs=1)
                nc.vector.reciprocal(rrow[:], av[64:65, :])
                rb = rot.tile([64, 512], f32, tag="rb", bufs=1)
                nc.gpsimd.partition_broadcast(rb[:], rrow[:])
                off = 64 * (h % 2)
                if debug and h == 0 and n == 0:
                    avc = rot.tile([P, 512], f32, tag="yts", bufs=2)
                    nc.vector.tensor_copy(avc[0:65, :], av[0:65, :])
                    dbg("av0", avc[:], [P, 512], f32)
                    dbg("rrow0", rrow[:], [1, 512], f32)
                nc.vector.tensor_tensor(ot[off:off + 64, h // 2, cols],
                                        av[0:64, :], rb[:], OP.mult)

            def wo_m(m, n):
                """output projection chunk m, token-half n + residual add."""
                cols = half_cols(n)
                ps = psum.tile([P, 512], f32, tag="work", bufs=3)
                for cp in range(CP):
                    nc.tensor.matmul(ps[:], wor[:, m, cp, :, :], ot[:, 2 * cp:2 * cp + 2, cols],
                                     start=(cp == 0), stop=(cp == CP - 1),
                                     perf_mode=PM.DoubleRow)
                tmp = rot.tile([P, 512], bf16, tag="wotmp", bufs=2)
                nc.scalar.activation(tmp[:], ps[:], AF.Identity,
                                     bias=bo_sb[:, m:m + 1], scale=io)
                nc.vector.tensor_tensor(x[:, m, cols], tmp[:], x[:, m, cols], OP.add)

            def conv_m(m, lo, hi, relu_act=False):
                """conv ensemble for out-chunk m, output cols [lo, hi)."""
                w = hi - lo
                scs = []
                grp = {}
                for bi, taps in enumerate(BRANCH_TAPS):
                    cps = psum.tile([P, 512], f32, tag="work", bufs=3)
                    first = True
                    for (tp, shift) in taps:
                        gi, tl = tp // 6, tp % 6
                        if gi not in grp:
                            wg = rot.tile([P, 6, CP, 2, P], fp8, tag="wcv", bufs=2, name="wg")
                            nc.sync.dma_start(wg[:], cw_d[gi, m])
                            grp[gi] = wg
                        u0 = 2 + shift + lo
                        for cp in range(CP):
                            nc.tensor.matmul(cps[:, 0:w], grp[gi][:, tl, cp, :, :],
                                             act2[:, 2 * cp:2 * cp + 2, u0:u0 + w],
                                             start=first, stop=(cp == CP - 1 and tp == taps[-1][0]),
                                             perf_mode=PM.DoubleRow)
                            first = False
                    sc_t = rot.tile([P, 512], bf16, tag=f"sc{bi}", bufs=2)
                    if relu_act:
                        nc.scalar.activation(sc_t[:, 0:w], cps[:, 0:w], AF.Relu,
                                             bias=cb_sb[:, bi, m:m + 1], scale=1.0)
                    else:
                        nc.vector.scalar_tensor_tensor(sc_t[:, 0:w], cps[:, 0:w], cb_sb[:, bi, m:m + 1],
                                                       zeros[:, 0:w], op0=OP.add, op1=OP.max)
                    scs.append(sc_t)
                t1 = rot.tile([P, 512], bf16, tag="cmb1", bufs=1)
                nc.gpsimd.tensor_tensor(t1[:, 0:w], scs[0][:, 0:w], scs[1][:, 0:w], OP.add)
                t2 = rot.tile([P, 512], bf16, tag="cmb2", bufs=1)
                nc.gpsimd.tensor_tensor(t2[:, 0:w], t1[:, 0:w], scs[2][:, 0:w], OP.add)
                nc.vector.scalar_tensor_tensor(x[:, m, lo:hi], t2[:, 0:w], icc_sb[:, m:m + 1],
                                               x[:, m, lo:hi], op0=OP.mult, op1=OP.add)

            # ---------------- PSUM layout ----------------

            # ---------------- emission ----------------
            # Interleaved schedule: while ACT streams exp for one query-half,
            # PE runs the other half's AV/conv/LN/projections and the next
            # layer's lead-in.  pexp is split into lo/hi key-half tiles;
            # pxAlo has bufs=H because the next layer's lo-exps (p1) stay
            # live until the AVs in the p2 loop.
            pex = {}

            def new_px(key, tag, bufs):
                t = rot.tile([P, CH // 2, 512], fp8, tag=tag, bufs=bufs, name="px")
                pex[key] = t
                return t

            # ---- layer-0 lead-in ----
            stA = ln_stats(0)
            stB = ln_stats(1)
            iA, mA = ln_rows(stA)
            ln_apply(0, iA, mA, act1, 0)
            iB, mB = ln_rows(stB)
            ln_apply(1, iB, mB, act1, 0)
            dbg("act1", act1[:], [P, CH, S], fp8)
            dbg("ivA", iA[:], [P, 512], bf16)
            dbg("mvA", mA[:], [P, 512], bf16)
            for m in range(CH):
                qkproj(m, 0, on_act=True)
                qkproj(m, 1, on_act=True)
            dbg("qt", qt[:], [P, 4, 2, S], fp8)
            dbg("kt", kt[:], [P, 4, 2, S], fp8)
            vt_of[0] = new_vt()
            for m in range(CH):
                for fh in range(2):
                    vproj(vt_of[0], m, fh)
            dbg("vt", vt_of[0][:], [P, CH, H * 80], fp8)
            # attn A of layer 0, AV lag-2
            for h in range(H):
                scores_exp(h, 0, 0, 4, new_px((0, 0, h, 0), "pxAlo", H))
                scores_exp(h, 0, 4, 8, new_px((0, 0, h, 1), "pxAhi", 2))
                if h == 0:
                    dbg("pexlo0", pex[(0, 0, 0, 0)][:], [P, CH // 2, 512], fp8)
                    dbg("pexhi0", pex[(0, 0, 0, 1)][:], [P, CH // 2, 512], fp8)
                if h >= 1:
                    av_norm(vt_of[0], h - 1, 0, pex[(0, 0, h - 1, 0)], pex[(0, 0, h - 1, 1)])
            av_norm(vt_of[0], H - 1, 0, pex[(0, 0, H - 1, 0)], pex[(0, 0, H - 1, 1)])
            dbg("otA", ot[:], [P, CH, S], fp8)
            for m in range(CH):
                wo_m(m, 0)
            dbg("xA", x[:], [P, CH, S], bf16)
            st2A = ln_stats(0)
            i2A, m2A = ln_rows(st2A)
            ln_apply(0, i2A, m2A, act2, 2)

            for l in range(NL):
                last = (l == NL - 1)
                # ---- B loop: scores/exp B + AV-B (lag-1) + conv-A cols [0:480)
                conv_m(0, 0, 480)
                conv_m(1, 0, 480)
                for h in range(H):
                    scores_exp(h, 1, 0, 4, new_px((l, 1, h, 0), "pxBlo", 2))
                    scores_exp(h, 1, 4, 8, new_px((l, 1, h, 1), "pxBhi", 2))
                    if h >= 1:
                        av_norm(vt_of[l], h - 1, 1, pex[(l, 1, h - 1, 0)], pex[(l, 1, h - 1, 1)])
                    if 1 <= h <= 6:
                        conv_m(h + 1, 0, 480)
                av_norm(vt_of[l], H - 1, 1, pex[(l, 1, H - 1, 0)], pex[(l, 1, H - 1, 1)])
                for m in range(CH):
                    wo_m(m, 1)
                st2B = ln_stats(1)
                i2B, m2B = ln_rows(st2B)
                ln_apply(1, i2B, m2B, act2, 2, edge=32)

                if not last:
                    # ---- A' p1 loop: conv seam + chunk2, LN1'A, proj'A,
                    # scores'/exp' keys 0:4 (all 16 pexp-lo tiles stay live)
                    vt_of[l + 1] = new_vt()
                    for h in range(H):
                        if h == 0:
                            for mm_ in range(CH):
                                conv_m(mm_, 480, 512)      # seam cols (needs act2 B edge)
                            st1A = ln_stats(0)
                            conv_m(0, 512, S)
                            i1A, m1A = ln_rows(st1A)
                            conv_m(1, 512, S)
                            ln_apply(0, i1A, m1A, act1, 0)
                        if h < CH:
                            qkproj(h, 0, on_act=True)
                        # scores for head h need qt'/kt' chunk m=h//2 <= h (emitted)
                        scores_exp(h, 0, 0, 4, new_px((l + 1, 0, h, 0), "pxAlo", H))
                        if 2 <= h <= 7:
                            conv_m(h, 512, S)              # cols [512:1024)

                    # ---- A' p2 loop: LN1'B + proj'B + vproj'4-7 + keys 4:8 + AV'
                    for h in range(H):
                        if h == 0:
                            st1B = ln_stats(1)
                            i1B, m1B = ln_rows(st1B)
                            ln_apply(1, i1B, m1B, act1, 0)
                        if h < CH:
                            qkproj(h, 1, on_act=True)
                        if h == 0:
                            for mm_ in range(CH):
                                vproj(vt_of[l + 1], mm_, 0)
                                vproj(vt_of[l + 1], mm_, 1)
                        scores_exp(h, 0, 4, 8, new_px((l + 1, 0, h, 1), "pxAhi", 2))
                        if h >= 1:
                            av_norm(vt_of[l + 1], h - 1, 0,
                                    pex[(l + 1, 0, h - 1, 0)], pex[(l + 1, 0, h - 1, 1)])
                    av_norm(vt_of[l + 1], H - 1, 0, pex[(l + 1, 0, H - 1, 0)], pex[(l + 1, 0, H - 1, 1)])
                    for m in range(CH):
                        wo_m(m, 0)
                    st2A_ = ln_stats(0)
                    i2A_, m2A_ = ln_rows(st2A_)
                    ln_apply(0, i2A_, m2A_, act2, 2)
                else:
                    # ---- final: conv seam + chunk2 + final LN (A overlapped)
                    for m in range(CH):
                        conv_m(m, 480, 512, relu_act=True)
                    stfA = ln_stats(0)
                    for m in range(CH):
                        conv_m(m, 512, S, relu_act=True)
                        if m == 0:
                            ifA, mfA = ln_rows(stfA)
                            lnf_apply(0, ifA, mfA)
                    stfB = ln_stats(1)
                    ifB, mfB = ln_rows(stfB)
                    lnf_apply(1, ifB, mfB)

    nc.compile()
    return nc, names


_BUILT = None


def _pow2_scale(arr, target=192.0):
    am = float(np.abs(arr).max())
    if am <= 0:
        return 1.0
    return 2.0 ** math.floor(math.log2(target / am))


def _pack_dr(W, k):
    """W [D_in, D_out] (already folded+scaled by caller except k) ->
    [P, CH_m, CP, 2, P] fp8 with element [p, m, cp, i, j] = k*W[256cp+128i+p, 128m+j]."""
    Wq = (W * k).astype(E4)
    return np.ascontiguousarray(
        Wq.reshape(CP, 2, P, CH, P).transpose(2, 3, 0, 1, 4))


# local column permutation inside each 128-out-chunk for Q/K so that the
# staging rows land shuffle-DMA-contiguous: [a_dk0:32, b_dk0:32, a_dk32:64, b_dk32:64]
_QK_PERM = np.concatenate([np.arange(0, 32), np.arange(64, 96),
                           np.arange(32, 64), np.arange(96, 128)])


def _perm_qk_cols(W):
    Wc = W.reshape(D, CH, P)[:, :, _QK_PERM]
    return np.ascontiguousarray(Wc.reshape(D, D))


def _perm_qk_bias(b):
    return np.ascontiguousarray(b.reshape(CH, P)[:, _QK_PERM].reshape(D))


def _pack_bias(b):
    return np.ascontiguousarray(b.reshape(CH, P).T)


def _prep(inputs):
    f = lambda kk: np.asarray(inputs[kk], np.float32)
    a1, b1 = f('ln1_a'), f('ln1_b')
    a2, b2 = f('ln2_a'), f('ln2_b')
    wq, wk, wv, wo = f('wq'), f('wk'), f('wv'), f('wo')
    bq, bk, bv, bo = f('bq'), f('bk'), f('bv'), f('bo')

    wq_e = _perm_qk_cols(a1[:, None] * wq / 8.0)
    bq_e = _perm_qk_bias((bq + b1 @ wq) / 8.0)
    wk_e = _perm_qk_cols(a1[:, None] * wk)
    bk_e = _perm_qk_bias(bk + b1 @ wk)
    wv_e = a1[:, None] * wv
    bv_e = bv + b1 @ wv

    sc = {'kq': _pow2_scale(wq_e), 'kk': _pow2_scale(wk_e),
          'kv': _pow2_scale(wv_e), 'ko': _pow2_scale(wo)}

    d = {}
    d['wq'] = _pack_dr(wq_e, sc['kq'])
    d['bq'] = _pack_bias(bq_e)
    d['wk'] = _pack_dr(wk_e, sc['kk'])
    d['bk'] = _pack_bias(bk_e)
    # V: moving-side pack [P, CP, 2, D]: [p, cp, i, f] = kv*Wv[256cp+128i+p, f]
    d['wv'] = np.ascontiguousarray(
        (wv_e * sc['kv']).astype(E4).reshape(CP, 2, P, D).transpose(2, 0, 1, 3))
    d['bv'] = bv_e.reshape(1, D)
    d['wo'] = _pack_dr(wo, sc['ko'])
    d['bo'] = _pack_bias(bo)

    # conv: fold BN + ensemble mean + ln2 affine
    Wf_all, bias_all = [], []
    for bi, fs in enumerate((5, 3, 1)):
        i = 3 - bi
        W = f(f'conv_w{i}')
        b = f(f'conv_b{i}')
        g, beta = f(f'bn_g{i}'), f(f'bn_b{i}')
        m, v = f(f'bn_m{i}'), f(f'bn_v{i}')
        s = g / np.sqrt(v + EPS_BN)
        Wf_all.append(W * s[:, None, None] * a2[None, :, None] / 3.0)
        bias_all.append((b + W.sum(axis=2) @ b2 - m) * s + beta)
    bias_all = [bb / 3.0 for bb in bias_all]
    sc['kc'] = 1.0  # per-channel scales via icc instead
    # per-output-channel pow2 scale over all branches/taps
    am = np.max(np.stack([np.abs(Wf).max(axis=(1, 2)) for Wf in Wf_all]), axis=0)
    kvec = 2.0 ** np.floor(np.log2(192.0 / np.maximum(am, 1e-20)))

    cw = np.empty((NTG, CH, P, 6, CP, 2, P), E4)
    cb = np.empty((P, 3, CH), np.float32)
    for bi in range(3):
        cb[:, bi, :] = _pack_bias(bias_all[bi] * kvec)
        ntap = len(BRANCH_TAPS[bi]) // 2
        for j in range(ntap):
            tp_hi = BRANCH_TAPS[bi][j][0]
            tp_lo = BRANCH_TAPS[bi][ntap + j][0]
            Wk = np.ascontiguousarray(Wf_all[bi][:, :, j].T) * kvec[None, :]
            hi8 = Wk.astype(E4)
            resid = Wk - hi8.astype(np.float32)
            cw[tp_hi // 6, :, :, tp_hi % 6] = _pack_dr(hi8.astype(np.float32), 1.0).transpose(1, 0, 2, 3, 4)
            cw[tp_lo // 6, :, :, tp_lo % 6] = _pack_dr(resid, 1.0).transpose(1, 0, 2, 3, 4)
    d['cw'] = cw
    d['cb'] = cb
    d['icc'] = _pack_bias(1.0 / kvec)
    return d, sc


def kernel(**inputs):
    global _BUILT
    shared, sc = _prep(inputs)
    if _BUILT is None:
        _BUILT = _build(sc)
    nc, names = _BUILT
    x = np.asarray(inputs['x'], np.float32)
    in_maps = []
    for b in range(N_CORES):
        m = {names[k]: v for k, v in shared.items()}
        m[names['xt']] = np.ascontiguousarray(x[b].T).astype(ml_dtypes.bfloat16)
        in_maps.append(m)
    res = run_bass_kernel_spmd(nc, in_maps, core_ids=list(range(N_CORES)))
    af = np.asarray(inputs['lnf_a'], np.float32)
    bf = np.asarray(inputs['lnf_b'], np.float32)
    out = np.empty((N_CORES, S, D), np.float32)
    for b in range(N_CORES):
        yt = res.results[b][names['yt']]
        out[b] = yt.T * af[None, :] + bf[None, :]
    return out
